# revision 5
# baseline (speedup 1.0000x reference)
"""Trainium2 Bass kernel for nn_CAModel (neural cellular automaton step).

Strategy: pure data-parallel over batch (16 samples -> 8 cores x 2).
v2 design (vs baseline): K=97 mm1 folds the full perceive into one matmul
(x + 2 shifted u-slabs + 3 shifted d-slabs + const row for b1), sobel is
only 4 DVE ops/sample; relu split across ScalarE and VectorE; mm2 stays
pixel-major (dense LDW+MM pairs issue at ~32ns); pools + life-mult on
GpSimd tensor_tensor; x/out in bf16 to halve HBM + DVE cost.
Host does layout transforms only (free); HW exec time is what counts.
"""

import numpy as np

# ---------------------------------------------------------------- constants
B, C, H, W = 16, 16, 256, 256
NCORES = 8
SPC = B // NCORES          # samples per core
HWPX = H * W               # 65536 pixels per sample
PITCH = 258                # padded row pitch (wrap col + 256 + wrap col)
NROWH = 34                 # rows -1..32 (halo top/bottom) for x_bf
XBF_F = NROWH * PITCH      # 8772
SOB_F = 32 * PITCH         # 8256 (rows 0..31 padded)
PIX_F = 8192               # 32*256 unpadded strip pixels
NT = HWPX // 128           # 512 pixel-tiles per sample
NSTRIP = 8                 # strips of 32 rows
CHUNK = 2048               # staging chunk (8 rows of a strip)
SUB = 1024                 # psum subchunk (one psh tile)
NCHUNK = HWPX // CHUNK     # 32 chunks per sample
KROWS = 97                 # mm1 contraction: 6*16 slabs + const row
RELU_ACT = 640             # cols of each 1024-subchunk relu'd on ScalarE
ALPHA_TH = 0.1
FIRE = 0.5

_BUILT = None


# ------------------------------------------------------------- host layouts
def _bf16():
    import ml_dtypes
    return ml_dtypes.bfloat16


def _prep_xbf(x):
    """x: [B, C, H, W] f32 -> [B, 128, XBF_F] bf16 strip layout w/ halo+wrap.

    partition p = hb*16 + c ; free = (r, pc): r = hl+1 for hl in -1..32,
    pc: 0 <-> w=255, 1..256 <-> w=0..255, 257 <-> w=0.   h = hb*32 + hl mod 256
    """
    bf16 = _bf16()
    xb = x.astype(bf16)                                   # [B, C, H, W]
    hidx = (np.arange(-1, 33)[None, :] + 32 * np.arange(8)[:, None]) % 256
    xr = xb[:, :, hidx, :]                                # [B, C, 8, 34, W]
    out = np.empty((B, 8, C, NROWH, PITCH), dtype=bf16)
    out[:, :, :, :, 1:257] = np.transpose(xr, (0, 2, 1, 3, 4))
    out[:, :, :, :, 0] = np.transpose(xr[:, :, :, :, 255], (0, 2, 1, 3))
    out[:, :, :, :, 257] = np.transpose(xr[:, :, :, :, 0], (0, 2, 1, 3))
    return np.ascontiguousarray(out.reshape(B, 128, XBF_F))


def _prep_xt(x):
    """x: [B, C, H, W] f32 -> pixel-major [B, 128, 8192] bf16.

    xt[b, p, 16*t + c] = x[b, c, pix] with pix = 128*t + p (raster order).
    """
    bf16 = _bf16()
    xf = x.reshape(B, C, HWPX).transpose(0, 2, 1)         # [B, pix, C]
    xf = xf.reshape(B, NT, 128, C).transpose(0, 2, 1, 3)  # [B, p, t, c]
    return np.ascontiguousarray(xf.reshape(B, 128, NT * C).astype(bf16))


def _prep_randt(rv):
    """rand_vals [B, 1, H, W] -> [B, 128, NT] f32, rt[b, p, t] = rv[b, pix]."""
    rf = rv.reshape(B, HWPX).reshape(B, NT, 128).transpose(0, 2, 1)
    return np.ascontiguousarray(rf.astype(np.float32))


def _unprep_out(op):
    """out_pm [B, 128, 8192] bf16 -> [B, C, H, W] f32."""
    o = op.astype(np.float32).reshape(B, 128, NT, C).transpose(0, 2, 1, 3)
    o = o.reshape(B, HWPX, C).transpose(0, 2, 1)
    return np.ascontiguousarray(o.reshape(B, C, H, W))


def _prep_weights(w1, b1, w2, b2):
    bf16 = _bf16()
    w1 = np.asarray(w1, np.float32)
    w2 = np.asarray(w2, np.float32)
    b1 = np.asarray(b1, np.float32)
    # S rows: [x; u(w+1); u(w-1); d(w-1); d(w); d(w+1); ones] with
    # u = x(h-1)+2x+x(h+1), d = x(h+1)-x(h-1).
    # pdx = 0.125*(u(w+1)-u(w-1)) ; pdy = 0.125 d(w-1)+0.25 d(w)+0.125 d(w+1)
    wid, wdx, wdy = w1[0::3], w1[1::3], w1[2::3]
    w1e = np.concatenate([
        wid, 0.125 * wdx, -0.125 * wdx,
        0.125 * wdy, 0.25 * wdy, 0.125 * wdy,
        b1.reshape(1, 128),
    ], axis=0)                                            # [97, 128]
    return (np.ascontiguousarray(w1e.astype(bf16)),
            np.ascontiguousarray(w2.astype(bf16)),
            np.asarray(b2, np.float32).reshape(16))


# ------------------------------------------------------------- build module
def _build(b2_nonzero):
    import concourse.bass as bass
    import concourse.bacc as bacc
    import concourse.mybir as mybir
    import concourse.tile as tile

    dt = mybir.dt
    op = mybir.AluOpType
    AF = mybir.ActivationFunctionType

    nc = bacc.Bacc("TRN2", target_bir_lowering=False, debug=False)

    xbf_d = nc.dram_tensor("xbf", (SPC, 128, XBF_F), dt.bfloat16, kind="ExternalInput")
    xt_d = nc.dram_tensor("xt", (SPC, 128, PIX_F), dt.bfloat16, kind="ExternalInput")
    rt_d = nc.dram_tensor("rt", (SPC, 128, NT), dt.float32, kind="ExternalInput")
    w1_d = nc.dram_tensor("w1e", (KROWS, 128), dt.bfloat16, kind="ExternalInput")
    w2_d = nc.dram_tensor("w2e", (128, 16), dt.bfloat16, kind="ExternalInput")
    b2_d = nc.dram_tensor("b2e", (1, 16), dt.float32, kind="ExternalInput")
    out_d = nc.dram_tensor("outp", (SPC, 128, PIX_F), dt.bfloat16, kind="ExternalOutput")

    with tile.TileContext(nc) as tc:
        with (
            tc.tile_pool(name="wpool", bufs=1) as wpool,
            tc.tile_pool(name="xbf", bufs=2) as p_xbf,
            tc.tile_pool(name="sob", bufs=2) as p_sob,
            tc.tile_pool(name="stage", bufs=3) as p_stage,
            tc.tile_pool(name="hsb", bufs=3) as p_hsb,
            tc.tile_pool(name="xt", bufs=2) as p_xt,
            tc.tile_pool(name="dxm", bufs=2) as p_dxm,
            tc.tile_pool(name="small", bufs=2) as p_small,
            tc.tile_pool(name="pscr", bufs=1) as p_pscr,
            tc.tile_pool(name="psh", bufs=3, space=bass.MemorySpace.PSUM) as p_psh,
            tc.tile_pool(name="psdx", bufs=2, space=bass.MemorySpace.PSUM) as p_psdx,
        ):
            w1_sb = wpool.tile([KROWS, 128], dt.bfloat16, tag="w1")
            nc.sync.dma_start(w1_sb[:], w1_d.ap())
            w2_sb = wpool.tile([128, 16], dt.bfloat16, tag="w2")
            nc.sync.dma_start(w2_sb[:], w2_d.ap())
            ones_sb = wpool.tile([1, CHUNK], dt.bfloat16, tag="ones")
            nc.vector.memset(ones_sb[:], 1.0)
            # gpsimd ucode warmup (first TT call pays ~8us ucode load)
            gwarm = wpool.tile([128, 2], dt.bfloat16, tag="gwarm")
            nc.vector.memset(gwarm[:], 0.0)
            nc.gpsimd.tensor_tensor(gwarm[:, 0:1], gwarm[:, 0:1], gwarm[:, 1:2], op.mult)
            nc.gpsimd.tensor_tensor(gwarm[:, 0:1], gwarm[:, 0:1], gwarm[:, 1:2], op.add)
            b2_sb = None
            if b2_nonzero:
                b2_sb = wpool.tile([128, 16], dt.float32, tag="b2")
                nc.sync.dma_start(b2_sb[:], b2_d.ap().broadcast_to([128, 16]))

            def emit_head(s):
                """Loads, sobel partials (U, D), update mask, pre-life pool."""
                st = {}
                xbf = p_xbf.tile([128, XBF_F], dt.bfloat16, tag="xbf")
                nc.scalar.dma_start(xbf[:], xbf_d.ap()[s])
                xt = p_xt.tile([128, PIX_F], dt.bfloat16, tag="xt")
                nc.scalar.dma_start(xt[:], xt_d.ap()[s])
                rt = p_pscr.tile([128, NT], dt.float32, tag="rt")
                nc.sync.dma_start(rt[:], rt_d.ap()[s])
                xbf3 = xbf.rearrange("p (r q) -> p r q", q=PITCH)  # [128,34,258]
                x_up = xbf3[:, 0:32, :]
                x_mid = xbf3[:, 1:33, :]
                x_dn = xbf3[:, 2:34, :]

                U = p_sob.tile([128, SOB_F], dt.bfloat16, tag="U")
                D = p_sob.tile([128, SOB_F], dt.bfloat16, tag="D")
                X2 = p_sob.tile([128, SOB_F], dt.bfloat16, tag="X2")
                U3 = U.rearrange("p (r q) -> p r q", q=PITCH)
                D3 = D.rearrange("p (r q) -> p r q", q=PITCH)
                X23 = X2.rearrange("p (r q) -> p r q", q=PITCH)
                nc.vector.tensor_scalar(X23[:], x_mid, 2.0, None, op.mult)
                nc.vector.tensor_tensor(U3[:], x_up, x_dn, op.add)
                nc.vector.tensor_tensor(U3[:], U3[:], X23[:], op.add)
                nc.vector.tensor_tensor(D3[:], x_dn, x_up, op.subtract)

                um = p_small.tile([128, NT], dt.bfloat16, tag="um")
                nc.vector.tensor_scalar(um[:], rt[:], FIRE, None, op.is_lt)

                xt3 = xt.rearrange("p (t c) -> p t c", c=C)
                alphaP = p_small.tile([128, NT], dt.bfloat16, tag="alP")
                preM = p_small.tile([128, NT], dt.bfloat16, tag="preM")
                nc.vector.tensor_copy(alphaP[:], xt3[:, :, 3])
                _pool_and_thresh(nc, p_pscr, alphaP, preM, op, dt)
                st.update(xbf=xbf, xt=xt, xt3=xt3, U=U, D=D, um=um, preM=preM)
                return st

            def emit_mid(s, st):
                """Per-chunk staging, mm1, relu, mm2, psdx evac + x update."""
                xbf, U, D = st["xbf"], st["U"], st["D"]
                xt, xt3, um = st["xt"], st["xt3"], st["um"]
                alphaN = p_small.tile([128, NT], dt.bfloat16, tag="alN")
                atmp = p_small.tile([128, 32], dt.float32, tag="atmp")
                psdx = None
                for k in range(NCHUNK):
                    hb = k // 4            # strip
                    rr = (k % 4) * 8       # first row of chunk within strip
                    pp = slice(16 * hb, 16 * hb + 16)
                    S = p_stage.tile([KROWS, CHUNK], dt.bfloat16, tag="S")
                    S3 = S.rearrange("p (r w) -> p r w", w=W)
                    xbf3 = xbf.rearrange("p (r q) -> p r q", q=PITCH)
                    U3 = U.rearrange("p (r q) -> p r q", q=PITCH)
                    D3 = D.rearrange("p (r q) -> p r q", q=PITCH)
                    # x slab: rows rr..rr+8 of strip hb (xbf row = r+1, col +1)
                    nc.sync.dma_start(
                        S3[0:16], xbf3[pp, rr + 1:rr + 9, 1:257])
                    # u(w+1), u(w-1)
                    nc.gpsimd.dma_start(
                        S3[16:32], U3[pp, rr:rr + 8, 2:258])
                    nc.gpsimd.dma_start(
                        S3[32:48], U3[pp, rr:rr + 8, 0:256])
                    # d(w-1), d(w), d(w+1)
                    nc.scalar.dma_start(
                        S3[48:64], D3[pp, rr:rr + 8, 0:256])
                    nc.scalar.dma_start(
                        S3[64:80], D3[pp, rr:rr + 8, 1:257])
                    nc.sync.dma_start(
                        S3[80:96], D3[pp, rr:rr + 8, 2:258])
                    nc.sync.dma_start(S[96:97, :], ones_sb[:])

                    for j in range(CHUNK // SUB):
                        sub0 = SUB * j
                        psh = p_psh.tile([128, SUB], dt.float32, tag="psh")
                        for i in range(2):
                            nc.tensor.matmul(
                                psh[:, 512 * i:512 * (i + 1)],
                                w1_sb[:],
                                S[:, sub0 + 512 * i:sub0 + 512 * (i + 1)])
                        hsb = p_hsb.tile([128, SUB], dt.bfloat16, tag="hsb")
                        nc.scalar.activation(hsb[:, 0:RELU_ACT],
                                             psh[:, 0:RELU_ACT], AF.Relu)
                        nc.vector.tensor_scalar(hsb[:, RELU_ACT:SUB],
                                                psh[:, RELU_ACT:SUB],
                                                0.0, None, op.max)
                        g = k * 2 + j      # global subchunk 0..63
                        if g % 4 == 0:
                            psdx = p_psdx.tile([128, 512], dt.float32, tag="psdx")
                        for t_loc in range(8):
                            tt = (g % 4) * 8 + t_loc
                            nc.tensor.matmul(
                                psdx[:, 16 * tt:16 * tt + 16],
                                hsb[:, 128 * t_loc:128 * (t_loc + 1)],
                                w2_sb[:])
                        if g % 4 == 3:
                            bk = g // 4    # psdx bank 0..15
                            DXM = p_dxm.tile([128, 512], dt.bfloat16, tag="DXM")
                            _evac_bank(nc, psdx, um, xt, xt3, DXM, alphaN,
                                       atmp, bk, b2_sb if b2_nonzero else None,
                                       op, dt)
                st["alphaN"] = alphaN

            def emit_tail(s, st):
                """Post-life pool, life mask, final multiply, store."""
                xt = st["xt"]
                postM = p_small.tile([128, NT], dt.bfloat16, tag="postM")
                _pool_and_thresh(nc, p_pscr, st["alphaN"], postM, op, dt)
                life = p_small.tile([128, NT], dt.bfloat16, tag="life")
                nc.gpsimd.tensor_tensor(life[:], st["preM"][:], postM[:], op.mult)
                nc.gpsimd.tensor_tensor(
                    xt.rearrange("p (t c) -> p t c", c=C),
                    xt.rearrange("p (t c) -> p t c", c=C),
                    life[:].broadcast_to([128, NT, C]), op.mult)
                nc.gpsimd.dma_start(out_d.ap()[s], xt[:])

            def _evac_bank(nc, psdx, um, xt, xt3, DXM, alphaN, atmp, bk,
                           b2_sb, op, dt):
                """One filled psdx bank (4096 px = 32 tiles): masked dx ->
                DXM (bf16), alphaN chunk, x += dx*um in place (bf16)."""
                ps3 = psdx.rearrange("p (t c) -> p t c", c=C)     # [128,32,16]
                umk = um[:, 32 * bk:32 * bk + 32]                 # [128, 32]
                if b2_sb is not None:
                    nc.vector.tensor_tensor(
                        ps3[:], ps3[:],
                        b2_sb[:].rearrange("p c -> p 1 c").broadcast_to([128, 32, C]),
                        op.add)
                dxm3 = DXM.rearrange("p (t c) -> p t c", c=C)
                nc.vector.tensor_tensor(dxm3, ps3[:],
                                        umk.broadcast_to([128, 32, C]), op.mult)
                nc.vector.tensor_tensor(atmp[:, 0:32], ps3[:, :, 3], umk, op.mult)
                nc.vector.tensor_tensor(alphaN[:, 32 * bk:32 * bk + 32],
                                        atmp[:, 0:32],
                                        xt3[:, 32 * bk:32 * bk + 32, 3], op.add)
                sl = slice(512 * bk, 512 * (bk + 1))
                nc.gpsimd.tensor_tensor(xt[:, sl], xt[:, sl], DXM[:, :], op.add)

            def _pool_and_thresh(nc, pool, alpha, outM, op, dt):
                """3x3 circular max-pool on pixel-major alpha [128, NT] then
                > ALPHA_TH.  TT-max ops on GpSimd, copies/threshold on DVE."""
                bf = dt.bfloat16
                aL = pool.tile([128, NT], bf, tag="aL")
                aR = pool.tile([128, NT], bf, tag="aR")
                nc.sync.dma_start(aL[1:128, :], alpha[0:127, :])
                nc.sync.dma_start(aR[0:127, :], alpha[1:128, :])
                eL = pool.tile([1, NT], bf, tag="eL")
                nc.sync.dma_start(eL[:], alpha[127:128, :])
                nc.vector.tensor_copy(aL[0:1, 0:NT:2], eL[0:1, 1:NT:2])
                nc.vector.tensor_copy(aL[0:1, 1:NT:2], eL[0:1, 0:NT - 1:2])
                edr = pool.tile([1, NT], bf, tag="edr")
                nc.vector.tensor_copy(edr[0:1, 0:NT:2], alpha[0:1, 1:NT:2])
                nc.vector.tensor_copy(edr[0:1, 1:NT:2], alpha[0:1, 0:NT - 1:2])
                nc.sync.dma_start(aR[127:128, :], edr[:])
                PW = pool.tile([128, NT], bf, tag="PW")
                nc.vector.tensor_tensor(PW[:], alpha[:, :], aL[:], op.max)
                nc.vector.tensor_tensor(PW[:], PW[:], aR[:], op.max)
                z2 = pool.tile([128, NT], bf, tag="z2")
                nc.vector.tensor_tensor(z2[:, 0:NT - 2], PW[:, 0:NT - 2],
                                        PW[:, 2:NT], op.max)
                nc.vector.tensor_tensor(outM[:, 2:NT - 2], z2[:, 0:NT - 4],
                                        PW[:, 4:NT], op.max)
                nc.vector.tensor_tensor(outM[:, 0:2], z2[:, 0:2],
                                        PW[:, NT - 2:NT], op.max)
                nc.vector.tensor_tensor(outM[:, NT - 2:NT], z2[:, NT - 4:NT - 2],
                                        PW[:, 0:2], op.max)
                nc.vector.tensor_scalar(outM[:], outM[:], ALPHA_TH, None, op.is_gt)

            # software-pipeline samples: next head overlaps current tail
            states = {}
            prev = None
            for s in range(SPC):
                states[s] = emit_head(s)
                if prev is not None:
                    emit_tail(prev, states.pop(prev))
                emit_mid(s, states[s])
                prev = s
            emit_tail(prev, states.pop(prev))

    nc.compile()
    return nc


def _get_built(b2_nonzero):
    global _BUILT
    if _BUILT is None or _BUILT[0] != b2_nonzero:
        _BUILT = (b2_nonzero, _build(b2_nonzero))
    return _BUILT[1]


# ------------------------------------------------------------------ kernel
def kernel(x, rand_vals, w1, b1, w2, b2):
    from concourse.bass_utils import run_bass_kernel_spmd

    x = np.asarray(x, np.float32)
    rand_vals = np.asarray(rand_vals, np.float32)
    w1e, w2e, b2e = _prep_weights(w1, b1, w2, b2)
    b2_nonzero = bool(np.any(b2e != 0.0))

    xbf = _prep_xbf(x)
    xt = _prep_xt(x)
    rt = _prep_randt(rand_vals)

    nc = _get_built(b2_nonzero)

    in_maps = []
    for i in range(NCORES):
        sl = slice(SPC * i, SPC * (i + 1))
        in_maps.append({
            "xbf": np.ascontiguousarray(xbf[sl]),
            "xt": np.ascontiguousarray(xt[sl]),
            "rt": np.ascontiguousarray(rt[sl]),
            "w1e": w1e, "w2e": w2e,
            "b2e": b2e.reshape(1, 16),
        })

    res = run_bass_kernel_spmd(nc, in_maps, core_ids=list(range(NCORES)))
    global LAST_RESULTS
    LAST_RESULTS = res
    outs = [res.results[i]["outp"] for i in range(NCORES)]
    out_pm = np.concatenate(outs, axis=0)        # [B, 128, 8192] bf16
    return _unprep_out(out_pm)
